# revision 1
# baseline (speedup 1.0000x reference)
"""Causal self-attention (B=4, T=2048, D=1024, H=16) on 8 TRN2 NeuronCores.

Sharding: tensor-parallel over 4 head-groups x data-parallel over 2 batch-groups.
Core c handles batches [2*(c//4), 2*(c//4)+2) and heads [4*(c%4), 4*(c%4)+4).
Each core computes a partial output projection (its 256 feature rows of W_proj);
the host sums the 4 head-group partials per batch group.

v2: all operands bf16 (fp32 PSUM accumulation), which buys:
 - x^T produced by X-bar DMA transpose straight from HBM (no PE transposes,
   no untransposed x load at all).
 - S computed per head PAIR with two row-tiled K=64 matmuls (heads live on
   partitions 0:64 / 64:128 of qt/kt, tile_position (0,0)/(64,0)) writing the
   two 512-halves of one [128,1024] psS tile; on HW the two row groups run
   concurrently.
 - ONE exp activation per key tile covers both heads; the causal diagonal is
   handled by slicing the exp at the q-offset plus [128,128] triangle-mask
   multiplies on DVE.
 - PV uses a 65-column stationary (64 V cols + ones col, set once) so the
   softmax denominator accumulates in psY row 64 for free; division is
   DVE reciprocal -> gpsimd partition_broadcast -> DVE multiply (PSUM -> yt
   directly, no PE broadcast matmul, no PE stall).
Weights are DMA'd per-dk chunk on the scalar queue while the first x
transposes run on the sync queue, so the first matmul issues ~1.5us in.
The output projection for q-block j is emitted after block j+1's S/PV so the
PE never waits on the divide chain.
"""
import functools
from contextlib import ExitStack

import numpy as np
import ml_dtypes

import concourse.bacc as bacc
import concourse.tile as tile
import concourse.mybir as mybir
from concourse.bass_utils import run_bass_kernel_spmd

F32 = mybir.dt.float32
BF16 = mybir.dt.bfloat16
EXP = mybir.ActivationFunctionType.Exp

B, T, D, H, HD = 4, 2048, 1024, 16, 64
NB, NH = 2, 4            # batches / heads per core
DL = NH * HD             # local feature dim (256)
NC = 8
WCOL = 768               # per-dk weight columns: Q(256) K(256) V(256) packed
NT5 = T // 512           # 4  (512-token q blocks)
NTT = T // 128           # 16 (128-token key tiles)
NDK = D // 128           # 8  (feature chunks of input dim)


@functools.lru_cache(maxsize=1)
def build():
    nc = bacc.Bacc("TRN2", target_bir_lowering=False, debug=False, num_devices=NC)
    x_d = nc.dram_tensor("x", [NB, T, D], BF16, kind="ExternalInput").ap()
    wqkv_d = nc.dram_tensor("wqkv", [D, WCOL], BF16, kind="ExternalInput").ap()
    wproj_d = nc.dram_tensor("wproj", [DL, D], BF16, kind="ExternalInput").ap()
    tri_d = nc.dram_tensor("tri", [128, 128], BF16, kind="ExternalInput").ap()
    out_d = nc.dram_tensor("out", [NB, T, D], BF16, kind="ExternalOutput").ap()

    with tile.TileContext(nc) as tc, ExitStack() as ctx:
        const = ctx.enter_context(tc.tile_pool(name="const", bufs=1))
        wpool = ctx.enter_context(tc.tile_pool(name="w", bufs=1))
        xt_pool = ctx.enter_context(tc.tile_pool(name="xt", bufs=1))

        # Emission order = DMA serialization order (the transpose deadlock
        # guard serializes DMA_TRANSPOSE against every other DMA): first x^T
        # chunk, then the weight chunks, then the remaining transposes; tri
        # last (not needed until the first diagonal mask ~60us in).
        xts = [xt_pool.tile([128, NDK, T], BF16, tag=f"xt{bb}", name=f"xt{bb}")
               for bb in range(NB)]
        nc.sync.dma_start_transpose(xts[0][:, :, 0:512], x_d[0, 0:512, :])

        wp_sb = wpool.tile([128, 2 * D], BF16)
        nc.scalar.dma_start(
            wp_sb[:].rearrange("p (a c) -> p a c", a=2),
            wproj_d.rearrange("(a p) c -> p a c", p=128))

        # weights in ONE DMA each so the transpose barrier chain stays
        # short: [w] -> [xbar0] -> [wp] -> [xbar1..7] -> [tri].
        w_sb = wpool.tile([128, NDK * WCOL], BF16)
        nc.scalar.dma_start(
            w_sb[:].rearrange("p (a c) -> p a c", a=NDK),
            wqkv_d.rearrange("(a p) c -> p a c", p=128))

        # V blocks per (key-tile ti, head h): 128 cols at (ti*NH+h)*128;
        # col 0 = ones (so the denominator lands in psY partition 0, where
        # reciprocal_approx_fast works - it breaks at base partition != 0),
        # cols 64:128 = V, cols 1:64 = zeros (psY rows 1:64 never read).
        v_sb = wpool.tile([128, NTT * NH * 128], BF16)
        v128 = v_sb[:].rearrange("p (n c) -> p n c", c=128)
        nc.gpsimd.memset(v_sb[:], 0.0)
        nc.gpsimd.memset(v128[:, :, 0:1], 1.0)

        # x^T via X-bar transpose: xt[p, dk, t] = x[b, t, dk*128+p].
        # One call per 512-token chunk keeps the DRAM side fully contiguous
        # (2KB rows). DMA_TRANSPOSE serializes against every other DMA
        # (deadlock guard), so all 8 transposes are emitted back-to-back
        # right after the small weight DMAs and before any output DMA.
        for b in range(NB):
            for t5 in range(NT5):
                if b == 0 and t5 == 0:
                    continue
                nc.sync.dma_start_transpose(
                    xts[b][:, :, 512 * t5:512 * (t5 + 1)],
                    x_d[b, 512 * t5:512 * (t5 + 1), :])

        tri = const.tile([128, 128], BF16)
        nc.scalar.dma_start(tri[:], tri_d)

        for b in range(NB):
            xt = xts[b]
            with tc.tile_pool(name="actv", bufs=1) as actv:
                # Q^T / K^T packed per head pair hp: rows 0:64 head 2hp,
                # rows 64:128 head 2hp+1, full T columns.
                qt = [actv.tile([128, T], BF16, tag=f"qt{cc}", name=f"qt{cc}")
                      for cc in range(2)]
                kt = [actv.tile([128, T], BF16, tag=f"kt{cc}", name=f"kt{cc}")
                      for cc in range(2)]

                # ---- Phase A: Q^T, K^T, V ----
                with tc.tile_pool(name="psQK", bufs=2, space="PSUM") as psQK, \
                     tc.tile_pool(name="psV", bufs=2, space="PSUM") as psV:
                    for t5 in range(NT5):
                        ts = slice(512 * t5, 512 * (t5 + 1))
                        for cc in range(2):     # Q^T
                            ps = psQK.tile([128, 512], F32, tag="qk")
                            for dk in range(NDK):
                                nc.tensor.matmul(
                                    ps[:],
                                    w_sb[:, dk * WCOL + cc * 128:dk * WCOL + cc * 128 + 128],
                                    xt[:, dk, ts],
                                    start=(dk == 0), stop=(dk == NDK - 1))
                            nc.vector.tensor_copy(qt[cc][:, ts], ps[:])
                        for cc in range(2):     # K^T
                            ps = psQK.tile([128, 512], F32, tag="qk")
                            for dk in range(NDK):
                                nc.tensor.matmul(
                                    ps[:],
                                    w_sb[:, dk * WCOL + 256 + cc * 128:dk * WCOL + 256 + cc * 128 + 128],
                                    xt[:, dk, ts],
                                    start=(dk == 0), stop=(dk == NDK - 1))
                            nc.vector.tensor_copy(kt[cc][:, ts], ps[:])
                        for tt in range(4):     # V (tokens stationary)
                            ps = psV.tile([128, 256], F32, tag="v")
                            for dk in range(NDK):
                                nc.tensor.matmul(
                                    ps[:],
                                    xt[:, dk, 512 * t5 + 128 * tt:512 * t5 + 128 * tt + 128],
                                    w_sb[:, dk * WCOL + 512:dk * WCOL + 768],
                                    start=(dk == 0), stop=(dk == NDK - 1))
                            ti = t5 * 4 + tt
                            nc.vector.tensor_copy(
                                v128[:, ti * NH:(ti + 1) * NH, 64:128],
                                ps[:].rearrange("p (n c) -> p n c", c=64))

                # ---- Phase B: attention + projection ----
                with tc.tile_pool(name="psS", bufs=2, space="PSUM") as psS_pool, \
                     tc.tile_pool(name="psY", bufs=2, space="PSUM") as psY_pool, \
                     tc.tile_pool(name="psO", bufs=2, space="PSUM") as psO_pool, \
                     tc.tile_pool(name="pP", bufs=17) as pP, \
                     tc.tile_pool(name="ytp", bufs=2) as ytp, \
                     tc.tile_pool(name="rcp", bufs=2) as rcp, \
                     tc.tile_pool(name="ost", bufs=2) as ost_pool:

                    def proj(j, yts):
                        # output projection for q-block j (yts from block j)
                        for g2 in range(2):
                            ostage = ost_pool.tile([128, 2 * D], BF16, tag="o")
                            for a in range(2):
                                tt = 2 * g2 + a
                                for nn2 in range(2):
                                    ps = psO_pool.tile([128, 512], F32, tag="o")
                                    for ff in range(2):
                                        nc.tensor.matmul(
                                            ps[:],
                                            yts[ff][:, 128 * tt:128 * tt + 128],
                                            wp_sb[:, ff * D + 512 * nn2:ff * D + 512 * nn2 + 512],
                                            start=(ff == 0), stop=(ff == 1))
                                    nc.vector.tensor_copy(
                                        ostage[:, a * D + 512 * nn2:a * D + 512 * nn2 + 512],
                                        ps[:])
                            nc.sync.dma_start(
                                out_d[b, 512 * j + 256 * g2:512 * j + 256 * g2 + 256]
                                .rearrange("(a p) c -> p a c", p=128),
                                ostage[:].rearrange("p (a c) -> p a c", a=2))

                    prev = None
                    for j in range(NT5):
                        yt = [ytp.tile([128, 512], BF16, tag=f"yt{ff}",
                                       name=f"yt{ff}") for ff in range(2)]
                        for hp in range(2):
                            qth, kth = qt[hp], kt[hp]
                            nk = 4 * j + 4
                            offs = [128 * (i - 4 * j) if i > 4 * j else 0
                                    for i in range(nk)]
                            Ps = []
                            for i in range(nk):
                                off = offs[i]
                                psS = psS_pool.tile([128, 1024], F32, tag="s")
                                P = pP.tile([128, 1024], BF16, tag="p")
                                Ps.append(P)
                                # two row-tiled K=64 matmuls (head pair)
                                nc.tensor.matmul(
                                    psS[:, off:512],
                                    kth[0:64, 128 * i:128 * i + 128],
                                    qth[0:64, 512 * j + off:512 * (j + 1)],
                                    start=True, stop=True)
                                nc.tensor.matmul(
                                    psS[:, 512 + off:1024],
                                    kth[64:128, 128 * i:128 * i + 128],
                                    qth[64:128, 512 * j + off:512 * (j + 1)],
                                    start=True, stop=True)
                                # one exp for both heads ([512+off-off:512] of
                                # the o-half below off is stale, never read)
                                nc.scalar.activation(
                                    P[:, off:1024], psS[:, off:1024], EXP,
                                    scale=0.125)
                                if i >= 4 * j:  # diagonal: causal triangle
                                    nc.vector.tensor_mul(
                                        P[:, off:off + 128],
                                        P[:, off:off + 128], tri[:])
                                    nc.vector.tensor_mul(
                                        P[:, 512 + off:512 + off + 128],
                                        P[:, 512 + off:512 + off + 128], tri[:])
                            for h01 in range(2):
                                h = 2 * hp + h01
                                psY = psY_pool.tile([128, 512], F32, tag="y")
                                for i in range(nk):
                                    off = offs[i]
                                    nc.tensor.matmul(
                                        psY[:, off:512],
                                        v_sb[:, 512 * i + 128 * h:512 * i + 128 * h + 128],
                                        Ps[i][:, h01 * 512 + off:h01 * 512 + 512],
                                        start=(i == 0), stop=(i == nk - 1))
                                # divide by the denominator (psY row 0)
                                rc = rcp.tile([1, 512], F32, tag="rc")
                                nc.vector.reciprocal_approx_fast(
                                    rc[:], psY[0:1, :])
                                rb = rcp.tile([128, 512], F32, tag="rb")
                                nc.gpsimd.partition_broadcast(rb[:], rc[:])
                                nc.vector.tensor_mul(
                                    yt[hp][64 * h01:64 * h01 + 64, :],
                                    psY[64:128, :], rb[64:128, :])
                        if prev is not None:
                            proj(*prev)
                        prev = (j, yt)
                    proj(*prev)

    nc.compile()
    return nc


def make_in_maps(x, W_qkv, W_proj):
    tri = np.triu(np.ones((128, 128), dtype=np.float32)).astype(ml_dtypes.bfloat16)
    in_maps = []
    for c in range(NC):
        bg, hg = c // 4, c % 4
        wq = np.concatenate(
            [W_qkv[:, 256 * hg:256 * hg + 256],
             W_qkv[:, 1024 + 256 * hg:1024 + 256 * hg + 256],
             W_qkv[:, 2048 + 256 * hg:2048 + 256 * hg + 256]], axis=1)
        in_maps.append({
            "x": np.ascontiguousarray(x[2 * bg:2 * bg + 2]).astype(ml_dtypes.bfloat16),
            "wqkv": wq.astype(ml_dtypes.bfloat16),
            "wproj": W_proj[256 * hg:256 * hg + 256, :].astype(ml_dtypes.bfloat16),
            "tri": tri,
        })
    return in_maps


def kernel(x, W_qkv, W_proj):
    x = np.asarray(x, dtype=np.float32)
    W_qkv = np.asarray(W_qkv, dtype=np.float32)
    W_proj = np.asarray(W_proj, dtype=np.float32)
    nc = build()
    res = run_bass_kernel_spmd(nc, make_in_maps(x, W_qkv, W_proj), list(range(NC)))
    out = np.zeros((B, T, D), dtype=np.float64)
    for c in range(NC):
        bg = c // 4
        out[2 * bg:2 * bg + 2] += res.results[c]["out"].astype(np.float64)
    return out.astype(np.float32)



# revision 6
# speedup vs baseline: 1.2069x; 1.2069x over previous
"""Causal self-attention (B=4, T=2048, D=1024, H=16) on 8 TRN2 NeuronCores.

Sharding: tensor-parallel over 4 head-groups x data-parallel over 2 batch-groups.
Core c handles batches [2*(c//4), 2*(c//4)+2) and heads [4*(c%4), 4*(c%4)+4).
Each core computes a partial output projection (its 256 feature rows of W_proj);
the host sums the 4 head-group partials per batch group (f32 partials).

v3: single software-pipelined stream over 8 blocks (2 batches x 4 q-blocks).
 - x^T is pre-transposed AND pre-cast to bf16 on the HOST (xt dram tensor
   [NB, NDK, 128, T]), so there are no DMA transposes at all: plain 1KB-row
   DMAs, first matmul issues ~5us in.
 - Block m = (b, j): S(j) for both head pairs, then proj(m-1), then the
   QKV phase-A chunk for block m+1, then PV(j). Phase-A/proj matmuls are
   interleaved between S tiles as PE fill while the activation engine
   (the near-bottleneck, ~190us of exp) drains the 2-slot psS pool.
 - PSUM: psS 2x[128,1024] (4 banks) + ONE shared 4-slot [128,512] pool
   (psB) used round-robin by psQK/psV (phase A), psY (PV+denominator),
   and psO (projection) - 8 banks total.
 - The projection result is staged per-psO-tile as bf16 (DVE copy) and
   DMA'd out; host sums bf16 partials in f64.
 - exp: head pair packed [off:1024-off] (h1 shifted down by off on
   diagonal tiles) so the activation covers exactly the useful columns.
 - PV uses the 65-column stationary (ones col 0 + V cols 64:128) so the
   softmax denominator accumulates in psY row 0 for free; division is
   DVE reciprocal -> gpsimd partition_broadcast -> DVE multiply.
"""
import functools
from contextlib import ExitStack

import numpy as np
import ml_dtypes

import concourse.bacc as bacc
import concourse.tile as tile
import concourse.mybir as mybir
from concourse.bass_utils import run_bass_kernel_spmd

F32 = mybir.dt.float32
BF16 = mybir.dt.bfloat16
EXP = mybir.ActivationFunctionType.Exp

B, T, D, H, HD = 4, 2048, 1024, 16, 64
NB, NH = 2, 4            # batches / heads per core
DL = NH * HD             # local feature dim (256)
NC = 8
WCOL = 768               # per-dk weight columns: Q(256) K(256) V(256) packed
NT5 = T // 512           # 4  (512-token q blocks)
NTT = T // 128           # 16 (128-token key tiles)
NDK = D // 128           # 8  (feature chunks of input dim)


@functools.lru_cache(maxsize=1)
def build():
    nc = bacc.Bacc("TRN2", target_bir_lowering=False, debug=False, num_devices=NC)
    xt_d = nc.dram_tensor("xt", [NB, NDK, 128, T], BF16, kind="ExternalInput").ap()
    wqkv_d = nc.dram_tensor("wqkv", [D, WCOL], BF16, kind="ExternalInput").ap()
    wproj_d = nc.dram_tensor("wproj", [DL, D], BF16, kind="ExternalInput").ap()
    tri_d = nc.dram_tensor("tri", [128, 128], BF16, kind="ExternalInput").ap()
    out_d = nc.dram_tensor("out", [NB, T, D], BF16, kind="ExternalOutput").ap()

    with tile.TileContext(nc) as tc, ExitStack() as ctx:
        const = ctx.enter_context(tc.tile_pool(name="const", bufs=1))
        wpool = ctx.enter_context(tc.tile_pool(name="w", bufs=1))
        xtp = ctx.enter_context(tc.tile_pool(name="xt", bufs=5))
        actv = ctx.enter_context(tc.tile_pool(name="actv", bufs=1))
        pP = ctx.enter_context(tc.tile_pool(name="pP", bufs=33))
        ytp = ctx.enter_context(tc.tile_pool(name="ytp", bufs=2))
        rcp = ctx.enter_context(tc.tile_pool(name="rcp", bufs=2))
        ostp = ctx.enter_context(tc.tile_pool(name="ostp", bufs=3))
        psS_pool = ctx.enter_context(tc.tile_pool(name="psS", bufs=2, space="PSUM"))
        psB = ctx.enter_context(tc.tile_pool(name="psB", bufs=4, space="PSUM"))

        wp_sb = wpool.tile([128, 2 * D], BF16)
        nc.scalar.dma_start(
            wp_sb[:].rearrange("p (a c) -> p a c", a=2),
            wproj_d.rearrange("(a p) c -> p a c", p=128))
        w_sb = wpool.tile([128, NDK * WCOL], BF16)
        nc.scalar.dma_start(
            w_sb[:].rearrange("p (a c) -> p a c", a=NDK),
            wqkv_d.rearrange("(a p) c -> p a c", p=128))
        tri = const.tile([128, 128], BF16)
        nc.scalar.dma_start(tri[:], tri_d)

        # x^T chunks: xc[(b,t5)][p, dk, t'] = x[b, 512*t5+t', dk*128+p].
        # Plain DMA from the host-pretransposed layout; 1KB rows.
        xcs = {}
        for b in range(NB):
            for t5 in range(NT5):
                xc = xtp.tile([128, NDK, 512], BF16, tag="xc", name=f"xc{b}{t5}")
                nc.sync.dma_start(
                    xc[:],
                    xt_d[b, :, :, 512 * t5:512 * (t5 + 1)]
                    .rearrange("a p t -> p a t"))
                xcs[(b, t5)] = xc

        # V blocks per (key-tile ti, head h): 128 cols at (ti*NH+h)*128;
        # col 0 = ones (denominator lands in psY row 0 where
        # reciprocal_approx_fast works), cols 64:128 = V, 1:64 = zeros.
        vs, v128s, qts, kts = [], [], [], []
        for b in range(NB):
            v_sb = actv.tile([128, NTT * NH * 128], BF16, tag=f"v{b}", name=f"v{b}")
            v128 = v_sb[:].rearrange("p (n c) -> p n c", c=128)
            nc.gpsimd.memset(v_sb[:], 0.0)
            nc.gpsimd.memset(v128[:, :, 0:1], 1.0)
            vs.append(v_sb)
            v128s.append(v128)
            qts.append([actv.tile([128, T], BF16, tag=f"qt{b}{cc}", name=f"qt{b}{cc}")
                        for cc in range(2)])
            kts.append([actv.tile([128, T], BF16, tag=f"kt{b}{cc}", name=f"kt{b}{cc}")
                        for cc in range(2)])

        def phA_groups(b, t5):
            """Phase-A fill groups for chunk (b, t5): 2 QT + 2 KT + 4 V."""
            xc = xcs[(b, t5)]
            ts = slice(512 * t5, 512 * (t5 + 1))

            def qk(cc, base, dst):
                def emit():
                    ps = psB.tile([128, 512], F32, tag="b5", name="psQK")
                    for dk in range(NDK):
                        nc.tensor.matmul(
                            ps[:],
                            w_sb[:, dk * WCOL + base + cc * 128:
                                 dk * WCOL + base + cc * 128 + 128],
                            xc[:, dk, :],
                            start=(dk == 0), stop=(dk == NDK - 1))
                    nc.vector.tensor_copy(dst[cc][:, ts], ps[:])
                return emit

            def vv(tt):
                def emit():
                    ps = psB.tile([128, 512], F32, tag="b5", name="psV")
                    for dk in range(NDK):
                        nc.tensor.matmul(
                            ps[:, 0:256],
                            xc[:, dk, 128 * tt:128 * tt + 128],
                            w_sb[:, dk * WCOL + 512:dk * WCOL + 768],
                            start=(dk == 0), stop=(dk == NDK - 1))
                    ti = t5 * 4 + tt
                    nc.vector.tensor_copy(
                        v128s[b][:, ti * NH:(ti + 1) * NH, 64:128],
                        ps[:, 0:256].rearrange("p (n c) -> p n c", c=64))
                return emit

            return ([qk(cc, 0, qts[b]) for cc in range(2)]
                    + [qk(cc, 256, kts[b]) for cc in range(2)]
                    + [vv(tt) for tt in range(4)])

        def proj_groups(b, j, yts):
            """Projection fill groups for q-block (b, j): 8 psO tiles,
            each 2 matmuls + a direct PSUM->DRAM f32 DMA."""
            def po(tt, nn2):
                def emit():
                    ps = psB.tile([128, 512], F32, tag="b5", name="psO")
                    for ff in range(2):
                        nc.tensor.matmul(
                            ps[:],
                            yts[ff][:, 128 * tt:128 * tt + 128],
                            wp_sb[:, ff * D + 512 * nn2:ff * D + 512 * nn2 + 512],
                            start=(ff == 0), stop=(ff == 1))
                    ost = ostp.tile([128, 512], BF16, tag="o", name="ost")
                    nc.vector.tensor_copy(ost[:], ps[:])
                    nc.sync.dma_start(
                        out_d[b, 512 * j + 128 * tt:512 * j + 128 * tt + 128,
                              512 * nn2:512 * nn2 + 512],
                        ost[:])
                return emit
            return [po(tt, nn2) for tt in range(4) for nn2 in range(2)]

        prev = None   # (b, j, yts) of previous block
        for m in range(2 * NT5):
            b, j = divmod(m, NT5)
            nk = 4 * j + 4
            offs = [128 * (i - 4 * j) if i > 4 * j else 0 for i in range(nk)]

            # fill queue: projection of block m-1, then phase A of block m+1
            fills = []
            if prev is not None:
                fills += proj_groups(*prev)
            if m + 1 < 2 * NT5:
                b2, j2 = divmod(m + 1, NT5)
                fills += phA_groups(b2, j2)
            if m == 0:
                # prologue: phase A chunk 0 emitted before anything else
                for g in phA_groups(0, 0):
                    g()

            # ---- S + exp for both head pairs, fills interleaved ----
            Ps = {}
            fi = 0
            for hp in range(2):
                qth, kth = qts[b][hp], kts[b][hp]
                for i in range(nk):
                    off = offs[i]
                    psS = psS_pool.tile([128, 1024], F32, tag="s", name="psS")
                    P = pP.tile([128, 1024], BF16, tag="p", name="P")
                    Ps[(hp, i)] = P
                    nc.tensor.matmul(
                        psS[:, off:512],
                        kth[0:64, 128 * i:128 * i + 128],
                        qth[0:64, 512 * j + off:512 * (j + 1)],
                        start=True, stop=True)
                    nc.tensor.matmul(
                        psS[:, 512:1024 - off],
                        kth[64:128, 128 * i:128 * i + 128],
                        qth[64:128, 512 * j + off:512 * (j + 1)],
                        start=True, stop=True)
                    nc.scalar.activation(
                        P[:, off:1024 - off], psS[:, off:1024 - off], EXP,
                        scale=0.125)
                    if i >= 4 * j:  # diagonal: causal triangle on both heads
                        nc.vector.tensor_mul(
                            P[:, off:off + 128], P[:, off:off + 128], tri[:])
                        nc.vector.tensor_mul(
                            P[:, 512:640], P[:, 512:640], tri[:])
                    # interleave one fill group per S tile
                    if fi < len(fills):
                        fills[fi]()
                        fi += 1
            while fi < len(fills):
                fills[fi]()
                fi += 1

            # ---- PV + normalize ----
            yt = [ytp.tile([128, 512], BF16, tag=f"yt{ff}", name=f"yt{ff}")
                  for ff in range(2)]
            for hp in range(2):
                for h01 in range(2):
                    h = 2 * hp + h01
                    psY = psB.tile([128, 512], F32, tag="b5", name="psY")
                    for i in range(nk):
                        off = offs[i]
                        mv = (Ps[(hp, i)][:, off:512] if h01 == 0
                              else Ps[(hp, i)][:, 512:1024 - off])
                        nc.tensor.matmul(
                            psY[:, off:512],
                            vs[b][:, 512 * i + 128 * h:512 * i + 128 * h + 128],
                            mv,
                            start=(i == 0), stop=(i == nk - 1))
                    rc = rcp.tile([1, 512], F32, tag="rc", name="rc")
                    nc.vector.reciprocal_approx_fast(rc[:], psY[0:1, :])
                    rb = rcp.tile([128, 512], F32, tag="rb", name="rb")
                    nc.gpsimd.partition_broadcast(rb[:], rc[:])
                    nc.vector.tensor_mul(
                        yt[hp][64 * h01:64 * h01 + 64, :],
                        psY[64:128, :], rb[64:128, :])
            prev = (b, j, yt)

        for g in proj_groups(*prev):   # epilogue: last block's projection
            g()

    nc.compile()
    return nc


def make_in_maps(x, W_qkv, W_proj):
    tri = np.triu(np.ones((128, 128), dtype=np.float32)).astype(ml_dtypes.bfloat16)
    xts = []
    for bg in range(2):
        xb = np.ascontiguousarray(x[2 * bg:2 * bg + 2]).astype(ml_dtypes.bfloat16)
        # [2, T, D] -> [2, NDK, 128, T]
        xts.append(np.ascontiguousarray(
            xb.reshape(NB, T, NDK, 128).transpose(0, 2, 3, 1)))
    in_maps = []
    for c in range(NC):
        bg, hg = c // 4, c % 4
        wq = np.concatenate(
            [W_qkv[:, 256 * hg:256 * hg + 256],
             W_qkv[:, 1024 + 256 * hg:1024 + 256 * hg + 256],
             W_qkv[:, 2048 + 256 * hg:2048 + 256 * hg + 256]], axis=1)
        in_maps.append({
            "xt": xts[bg],
            "wqkv": wq.astype(ml_dtypes.bfloat16),
            "wproj": W_proj[256 * hg:256 * hg + 256, :].astype(ml_dtypes.bfloat16),
            "tri": tri,
        })
    return in_maps


def kernel(x, W_qkv, W_proj):
    x = np.asarray(x, dtype=np.float32)
    W_qkv = np.asarray(W_qkv, dtype=np.float32)
    W_proj = np.asarray(W_proj, dtype=np.float32)
    nc = build()
    res = run_bass_kernel_spmd(nc, make_in_maps(x, W_qkv, W_proj), list(range(NC)))
    out = np.zeros((B, T, D), dtype=np.float64)
    for c in range(NC):
        bg = c // 4
        out[2 * bg:2 * bg + 2] += res.results[c]["out"].astype(np.float64)
    return out.astype(np.float32)


# revision 10
# speedup vs baseline: 1.3175x; 1.0916x over previous
"""Causal self-attention (B=4, T=2048, D=1024, H=16) on 8 TRN2 NeuronCores.

Sharding: tensor-parallel over 4 head-groups x data-parallel over 2 batch-groups.
Core c handles batches [2*(c//4), 2*(c//4)+2) and heads [4*(c%4), 4*(c%4)+4).
Each core computes a partial output projection (its 256 feature rows of W_proj);
the host sums the 4 head-group partials per batch group (f32 partials).

v3: single software-pipelined stream over 8 blocks (2 batches x 4 q-blocks).
 - x^T is pre-transposed AND pre-cast to bf16 on the HOST (xt dram tensor
   [NB, NDK, 128, T]), so there are no DMA transposes at all: plain 1KB-row
   DMAs, first matmul issues ~5us in.
 - Block m = (b, j): S(j) for both head pairs, then proj(m-1), then the
   QKV phase-A chunk for block m+1, then PV(j). Phase-A/proj matmuls are
   interleaved between S tiles as PE fill while the activation engine
   (the near-bottleneck, ~190us of exp) drains the 2-slot psS pool.
 - PSUM: psS 2x[128,1024] (4 banks) + ONE shared 4-slot [128,512] pool
   (psB) used round-robin by psQK/psV (phase A), psY (PV+denominator),
   and psO (projection) - 8 banks total.
 - The projection result is staged per-psO-tile as bf16 (DVE copy) and
   DMA'd out; host sums bf16 partials in f64.
 - exp: head pair packed [off:1024-off] (h1 shifted down by off on
   diagonal tiles) so the activation covers exactly the useful columns.
 - PV uses the 65-column stationary (ones col 0 + V cols 64:128) so the
   softmax denominator accumulates in psY row 0 for free; division is
   DVE reciprocal -> gpsimd partition_broadcast -> DVE multiply.
"""
import functools
from contextlib import ExitStack

import numpy as np
import ml_dtypes

import concourse.bacc as bacc
import concourse.tile as tile
import concourse.mybir as mybir
from concourse.bass_utils import run_bass_kernel_spmd

F32 = mybir.dt.float32
BF16 = mybir.dt.bfloat16
EXP = mybir.ActivationFunctionType.Exp

B, T, D, H, HD = 4, 2048, 1024, 16, 64
NB, NH = 2, 4            # batches / heads per core
DL = NH * HD             # local feature dim (256)
NC = 8
WCOL = 768               # per-dk weight columns: Q(256) K(256) V(256) packed
NT5 = T // 512           # 4  (512-token q blocks)
NTT = T // 128           # 16 (128-token key tiles)
NDK = D // 128           # 8  (feature chunks of input dim)


@functools.lru_cache(maxsize=1)
def build():
    nc = bacc.Bacc("TRN2", target_bir_lowering=False, debug=False, num_devices=NC)
    xt_d = nc.dram_tensor("xt", [NB, NDK, 128, T], BF16, kind="ExternalInput").ap()
    wqkv_d = nc.dram_tensor("wqkv", [D, WCOL], BF16, kind="ExternalInput").ap()
    wproj_d = nc.dram_tensor("wproj", [DL, D], BF16, kind="ExternalInput").ap()
    tri_d = nc.dram_tensor("tri", [128, 128], BF16, kind="ExternalInput").ap()
    out_d = nc.dram_tensor("out", [NB, T, D], BF16, kind="ExternalOutput").ap()

    with tile.TileContext(nc) as tc, ExitStack() as ctx:
        const = ctx.enter_context(tc.tile_pool(name="const", bufs=1))
        wpool = ctx.enter_context(tc.tile_pool(name="w", bufs=1))
        xtp = ctx.enter_context(tc.tile_pool(name="xt", bufs=5))
        actv = ctx.enter_context(tc.tile_pool(name="actv", bufs=1))
        pP = ctx.enter_context(tc.tile_pool(name="pP", bufs=33))
        ytp = ctx.enter_context(tc.tile_pool(name="ytp", bufs=2))
        rcp = ctx.enter_context(tc.tile_pool(name="rcp", bufs=2))
        ostp = ctx.enter_context(tc.tile_pool(name="ostp", bufs=3))
        psS_pool = ctx.enter_context(tc.tile_pool(name="psS", bufs=2, space="PSUM"))
        psB = ctx.enter_context(tc.tile_pool(name="psB", bufs=4, space="PSUM"))

        # scalar-queue DMA order = criticality: w (first matmul), tri
        # (first diagonal mask ~25us), wp (first projection ~45us).
        w_sb = wpool.tile([128, NDK * WCOL], BF16)
        nc.scalar.dma_start(
            w_sb[:].rearrange("p (a c) -> p a c", a=NDK),
            wqkv_d.rearrange("(a p) c -> p a c", p=128))
        tri = const.tile([128, 128], BF16)
        nc.scalar.dma_start(tri[:], tri_d)
        wp_sb = wpool.tile([128, 2 * D], BF16)
        nc.scalar.dma_start(
            wp_sb[:].rearrange("p (a c) -> p a c", a=2),
            wproj_d.rearrange("(a p) c -> p a c", p=128))

        # x^T chunks: xc[(b,t5)][p, dk, t'] = x[b, 512*t5+t', dk*128+p].
        # Plain DMA from the host-pretransposed layout; 1KB rows. Issued
        # just-in-time (chunk m+2 during block m) so the first chunk + w
        # have the full HBM bandwidth at startup.
        xcs = {}

        def xc_dma(m1):
            b, t5 = divmod(m1, NT5)
            xc = xtp.tile([128, NDK, 512], BF16, tag="xc", name=f"xc{b}{t5}")
            nc.sync.dma_start(
                xc[:],
                xt_d[b, :, :, 512 * t5:512 * (t5 + 1)]
                .rearrange("a p t -> p a t"))
            xcs[(b, t5)] = xc

        xc_dma(0)
        xc_dma(1)

        # V blocks per (key-tile ti, head h): 128 cols at (ti*NH+h)*128;
        # col 0 = ones (denominator lands in psY row 0 where
        # reciprocal_approx_fast works), cols 64:128 = V, 1:64 = zeros.
        vs, v128s, qts, kts = [], [], [], []
        for b in range(NB):
            v_sb = actv.tile([128, NTT * NH * 128], BF16, tag=f"v{b}", name=f"v{b}")
            v128 = v_sb[:].rearrange("p (n c) -> p n c", c=128)
            # only cols 0:64 of each 128-block are ever read beyond the V
            # data: col 0 = ones, 1:64 = zeros. b=0 on DVE (ready early),
            # b=1 on gpsimd.
            eng = nc.vector if b == 0 else nc.gpsimd
            eng.memset(v128[:, :, 1:64], 0.0)
            eng.memset(v128[:, :, 0:1], 1.0)
            vs.append(v_sb)
            v128s.append(v128)
            qts.append([actv.tile([128, T], BF16, tag=f"qt{b}{cc}", name=f"qt{b}{cc}")
                        for cc in range(2)])
            kts.append([actv.tile([128, T], BF16, tag=f"kt{b}{cc}", name=f"kt{b}{cc}")
                        for cc in range(2)])

        def phA_groups(b, t5):
            """Phase-A fill groups for chunk (b, t5): 2 QT + 2 KT + 4 V."""
            xc = xcs[(b, t5)]
            ts = slice(512 * t5, 512 * (t5 + 1))

            def qk(cc, base, dst):
                def emit():
                    ps = psB.tile([128, 512], F32, tag="b5", name="psQK")
                    for dk in range(NDK):
                        nc.tensor.matmul(
                            ps[:],
                            w_sb[:, dk * WCOL + base + cc * 128:
                                 dk * WCOL + base + cc * 128 + 128],
                            xc[:, dk, :],
                            start=(dk == 0), stop=(dk == NDK - 1))
                    nc.vector.tensor_copy(dst[cc][:, ts], ps[:])
                return emit

            def vv(tt):
                def emit():
                    ps = psB.tile([128, 512], F32, tag="b5", name="psV")
                    for dk in range(NDK):
                        nc.tensor.matmul(
                            ps[:, 0:256],
                            xc[:, dk, 128 * tt:128 * tt + 128],
                            w_sb[:, dk * WCOL + 512:dk * WCOL + 768],
                            start=(dk == 0), stop=(dk == NDK - 1))
                    ti = t5 * 4 + tt
                    nc.vector.tensor_copy(
                        v128s[b][:, ti * NH:(ti + 1) * NH, 64:128],
                        ps[:, 0:256].rearrange("p (n c) -> p n c", c=64))
                return emit

            return ([qk(cc, 0, qts[b]) for cc in range(2)]
                    + [qk(cc, 256, kts[b]) for cc in range(2)]
                    + [vv(tt) for tt in range(4)])

        def proj_groups(b, j, yts):
            """Projection fill groups for q-block (b, j): 8 psO tiles,
            each 2 matmuls + a direct PSUM->DRAM f32 DMA."""
            def po(tt, nn2):
                def emit():
                    ps = psB.tile([128, 512], F32, tag="b5", name="psO")
                    for ff in range(2):
                        nc.tensor.matmul(
                            ps[:],
                            yts[ff][:, 128 * tt:128 * tt + 128],
                            wp_sb[:, ff * D + 512 * nn2:ff * D + 512 * nn2 + 512],
                            start=(ff == 0), stop=(ff == 1))
                    ost = ostp.tile([128, 512], BF16, tag="o", name="ost")
                    nc.vector.tensor_copy(ost[:], ps[:])
                    nc.sync.dma_start(
                        out_d[b, 512 * j + 128 * tt:512 * j + 128 * tt + 128,
                              512 * nn2:512 * nn2 + 512],
                        ost[:])
                return emit
            return [po(tt, nn2) for tt in range(4) for nn2 in range(2)]

        prev = None   # (b, j, yts) of previous block
        for m in range(2 * NT5):
            b, j = divmod(m, NT5)
            nk = 4 * j + 4
            offs = [128 * (i - 4 * j) if i > 4 * j else 0 for i in range(nk)]

            if m + 2 < 2 * NT5:
                xc_dma(m + 2)
            # fill queue: phase A of block m+1 first (its deps are long
            # satisfied), then the projection of block m-1 (whose yt
            # divides drain on DVE during the first fills).
            fills = []
            if m + 1 < 2 * NT5:
                b2, j2 = divmod(m + 1, NT5)
                fills += phA_groups(b2, j2)
            if prev is not None:
                fills += proj_groups(*prev)
            if m == 0:
                # prologue: phase A chunk 0 emitted before anything else
                for g in phA_groups(0, 0):
                    g()

            # ---- S + exp for both head pairs, fills interleaved ----
            Ps = {}
            fi = 0
            for hp in range(2):
                qth, kth = qts[b][hp], kts[b][hp]
                for i in range(nk):
                    off = offs[i]
                    psS = psS_pool.tile([128, 1024], F32, tag="s", name="psS")
                    P = pP.tile([128, 1024], BF16, tag="p", name="P")
                    Ps[(hp, i)] = P
                    nc.tensor.matmul(
                        psS[:, off:512],
                        kth[0:64, 128 * i:128 * i + 128],
                        qth[0:64, 512 * j + off:512 * (j + 1)],
                        start=True, stop=True)
                    nc.tensor.matmul(
                        psS[:, 512:1024 - off],
                        kth[64:128, 128 * i:128 * i + 128],
                        qth[64:128, 512 * j + off:512 * (j + 1)],
                        start=True, stop=True)
                    nc.scalar.activation(
                        P[:, off:1024 - off], psS[:, off:1024 - off], EXP,
                        scale=0.125)
                    if i >= 4 * j:  # diagonal: causal triangle on both heads
                        nc.vector.tensor_mul(
                            P[:, off:off + 128], P[:, off:off + 128], tri[:])
                        nc.vector.tensor_mul(
                            P[:, 512:640], P[:, 512:640], tri[:])
                    # interleave one fill group per S tile
                    if fi < len(fills):
                        fills[fi]()
                        fi += 1
            while fi < len(fills):
                fills[fi]()
                fi += 1

            # ---- PV + normalize ----
            yt = [ytp.tile([128, 512], BF16, tag=f"yt{ff}", name=f"yt{ff}")
                  for ff in range(2)]
            for hp in range(2):
                for h01 in range(2):
                    h = 2 * hp + h01
                    psY = psB.tile([128, 512], F32, tag="b5", name="psY")
                    for i in range(nk):
                        off = offs[i]
                        mv = (Ps[(hp, i)][:, off:512] if h01 == 0
                              else Ps[(hp, i)][:, 512:1024 - off])
                        nc.tensor.matmul(
                            psY[:, off:512],
                            vs[b][:, 512 * i + 128 * h:512 * i + 128 * h + 128],
                            mv,
                            start=(i == 0), stop=(i == nk - 1))
                    rc = rcp.tile([1, 512], F32, tag="rc", name="rc")
                    nc.vector.reciprocal_approx_fast(rc[:], psY[0:1, :])
                    rb = rcp.tile([128, 512], F32, tag="rb", name="rb")
                    nc.gpsimd.partition_broadcast(rb[:], rc[:])
                    nc.vector.tensor_mul(
                        yt[hp][64 * h01:64 * h01 + 64, :],
                        psY[64:128, :], rb[64:128, :])
            prev = (b, j, yt)

        for g in proj_groups(*prev):   # epilogue: last block's projection
            g()

    nc.compile()
    return nc


def make_in_maps(x, W_qkv, W_proj):
    tri = np.triu(np.ones((128, 128), dtype=np.float32)).astype(ml_dtypes.bfloat16)
    xts = []
    for bg in range(2):
        xb = np.ascontiguousarray(x[2 * bg:2 * bg + 2]).astype(ml_dtypes.bfloat16)
        # [2, T, D] -> [2, NDK, 128, T]
        xts.append(np.ascontiguousarray(
            xb.reshape(NB, T, NDK, 128).transpose(0, 2, 3, 1)))
    in_maps = []
    for c in range(NC):
        bg, hg = c // 4, c % 4
        wq = np.concatenate(
            [W_qkv[:, 256 * hg:256 * hg + 256],
             W_qkv[:, 1024 + 256 * hg:1024 + 256 * hg + 256],
             W_qkv[:, 2048 + 256 * hg:2048 + 256 * hg + 256]], axis=1)
        in_maps.append({
            "xt": xts[bg],
            "wqkv": wq.astype(ml_dtypes.bfloat16),
            "wproj": W_proj[256 * hg:256 * hg + 256, :].astype(ml_dtypes.bfloat16),
            "tri": tri,
        })
    return in_maps


def kernel(x, W_qkv, W_proj):
    x = np.asarray(x, dtype=np.float32)
    W_qkv = np.asarray(W_qkv, dtype=np.float32)
    W_proj = np.asarray(W_proj, dtype=np.float32)
    nc = build()
    res = run_bass_kernel_spmd(nc, make_in_maps(x, W_qkv, W_proj), list(range(NC)))
    out = np.zeros((B, T, D), dtype=np.float64)
    for c in range(NC):
        bg = c // 4
        out[2 * bg:2 * bg + 2] += res.results[c]["out"].astype(np.float64)
    return out.astype(np.float32)


# revision 12
# speedup vs baseline: 1.3561x; 1.0294x over previous
"""Causal self-attention (B=4, T=2048, D=1024, H=16) on 8 TRN2 NeuronCores.

Sharding: tensor-parallel over 4 head-groups x data-parallel over 2 batch-groups.
Core c handles batches [2*(c//4), 2*(c//4)+2) and heads [4*(c%4), 4*(c%4)+4).
Each core computes a partial output projection (its 256 feature rows of W_proj);
the host sums the 4 head-group partials per batch group (f32 partials).

v3: single software-pipelined stream over 8 blocks (2 batches x 4 q-blocks).
 - x^T is pre-transposed AND pre-cast to bf16 on the HOST (xt dram tensor
   [NB, NDK, 128, T]), so there are no DMA transposes at all: plain 1KB-row
   DMAs, first matmul issues ~5us in.
 - Block m = (b, j): S(j) for both head pairs, then proj(m-1), then the
   QKV phase-A chunk for block m+1, then PV(j). Phase-A/proj matmuls are
   interleaved between S tiles as PE fill while the activation engine
   (the near-bottleneck, ~190us of exp) drains the 2-slot psS pool.
 - PSUM: psS 2x[128,1024] (4 banks) + ONE shared 4-slot [128,512] pool
   (psB) used round-robin by psQK/psV (phase A), psY (PV+denominator),
   and psO (projection) - 8 banks total.
 - The projection result is staged per-psO-tile as bf16 (DVE copy) and
   DMA'd out; host sums bf16 partials in f64.
 - exp: head pair packed [off:1024-off] (h1 shifted down by off on
   diagonal tiles) so the activation covers exactly the useful columns.
 - PV uses the 65-column stationary (ones col 0 + V cols 64:128) so the
   softmax denominator accumulates in psY row 0 for free; division is
   DVE reciprocal -> gpsimd partition_broadcast -> DVE multiply.
"""
import functools
from contextlib import ExitStack

import numpy as np
import ml_dtypes

import concourse.bacc as bacc
import concourse.tile as tile
import concourse.mybir as mybir
from concourse.bass_utils import run_bass_kernel_spmd

F32 = mybir.dt.float32
BF16 = mybir.dt.bfloat16
EXP = mybir.ActivationFunctionType.Exp

B, T, D, H, HD = 4, 2048, 1024, 16, 64
NB, NH = 2, 4            # batches / heads per core
DL = NH * HD             # local feature dim (256)
NC = 8
WCOL = 768               # per-dk weight columns: Q(256) K(256) V(256) packed
NT5 = T // 512           # 4  (512-token q blocks)
NTT = T // 128           # 16 (128-token key tiles)
NDK = D // 128           # 8  (feature chunks of input dim)


@functools.lru_cache(maxsize=1)
def build():
    nc = bacc.Bacc("TRN2", target_bir_lowering=False, debug=False, num_devices=NC)
    xt_d = nc.dram_tensor("xt", [NB, NDK, 128, T], BF16, kind="ExternalInput").ap()
    wqkv_d = nc.dram_tensor("wqkv", [D, WCOL], BF16, kind="ExternalInput").ap()
    wproj_d = nc.dram_tensor("wproj", [DL, D], BF16, kind="ExternalInput").ap()
    tri_d = nc.dram_tensor("tri", [128, 128], BF16, kind="ExternalInput").ap()
    out_d = nc.dram_tensor("out", [NB, T, D], BF16, kind="ExternalOutput").ap()

    with tile.TileContext(nc) as tc, ExitStack() as ctx:
        const = ctx.enter_context(tc.tile_pool(name="const", bufs=1))
        wpool = ctx.enter_context(tc.tile_pool(name="w", bufs=1))
        xtp = ctx.enter_context(tc.tile_pool(name="xt", bufs=5))
        actv = ctx.enter_context(tc.tile_pool(name="actv", bufs=1))
        pP = ctx.enter_context(tc.tile_pool(name="pP", bufs=33))
        ytp = ctx.enter_context(tc.tile_pool(name="ytp", bufs=2))
        rcp = ctx.enter_context(tc.tile_pool(name="rcp", bufs=2))
        ostp = ctx.enter_context(tc.tile_pool(name="ostp", bufs=3))
        psS_pool = ctx.enter_context(tc.tile_pool(name="psS", bufs=2, space="PSUM"))
        psB = ctx.enter_context(tc.tile_pool(name="psB", bufs=4, space="PSUM"))

        # scalar-queue DMA order = criticality: w Q-cols (first matmul
        # chain), K-cols, V-cols, tri (first diagonal mask ~20us), wp
        # (first projection ~40us). Startup is HBM-bound across all 8
        # cores, so the first chain's deps are kept minimal.
        w_sb = wpool.tile([128, NDK * WCOL], BF16)
        w3 = w_sb[:].rearrange("p (a q c) -> p a q c", a=NDK, q=3)
        for q3 in range(3):
            nc.scalar.dma_start(
                w3[:, :, q3],
                wqkv_d.rearrange("(a p) (q c) -> p a q c", p=128, q=3)[:, :, q3])
        tri = const.tile([128, 128], BF16)
        nc.scalar.dma_start(tri[:], tri_d)
        wp_sb = wpool.tile([128, 2 * D], BF16)
        nc.scalar.dma_start(
            wp_sb[:].rearrange("p (a c) -> p a c", a=2),
            wproj_d.rearrange("(a p) c -> p a c", p=128))

        # x^T chunks: xc[(b,t5)][p, dk, t'] = x[b, 512*t5+t', dk*128+p].
        # Plain DMA from the host-pretransposed layout; 1KB rows. Issued
        # just-in-time (chunk m+2 during block m) so the first chunk + w
        # have the full HBM bandwidth at startup.
        xcs = {}

        def xc_dma(m1):
            b, t5 = divmod(m1, NT5)
            xc = xtp.tile([128, NDK, 512], BF16, tag="xc", name=f"xc{b}{t5}")
            nc.sync.dma_start(
                xc[:],
                xt_d[b, :, :, 512 * t5:512 * (t5 + 1)]
                .rearrange("a p t -> p a t"))
            xcs[(b, t5)] = xc

        xc_dma(0)
        xc_dma(1)

        # V blocks per (key-tile ti, head h): 128 cols at (ti*NH+h)*128;
        # col 0 = ones (denominator lands in psY row 0 where
        # reciprocal_approx_fast works), cols 64:128 = V, 1:64 = zeros.
        vs, v128s, qts, kts = [], [], [], []
        for b in range(NB):
            v_sb = actv.tile([128, NTT * NH * 128], BF16, tag=f"v{b}", name=f"v{b}")
            v128 = v_sb[:].rearrange("p (n c) -> p n c", c=128)
            # only cols 0:64 of each 128-block are ever read beyond the V
            # data: col 0 = ones, 1:64 = zeros. b=0 on DVE (ready early),
            # b=1 on gpsimd.
            eng = nc.vector if b == 0 else nc.gpsimd
            eng.memset(v128[:, :, 1:64], 0.0)
            eng.memset(v128[:, :, 0:1], 1.0)
            vs.append(v_sb)
            v128s.append(v128)
            qts.append([actv.tile([128, T], BF16, tag=f"qt{b}{cc}", name=f"qt{b}{cc}")
                        for cc in range(2)])
            kts.append([actv.tile([128, T], BF16, tag=f"kt{b}{cc}", name=f"kt{b}{cc}")
                        for cc in range(2)])

        def phA_groups(b, t5):
            """Phase-A fill groups for chunk (b, t5): 2 QT + 2 KT + 4 V."""
            xc = xcs[(b, t5)]
            ts = slice(512 * t5, 512 * (t5 + 1))

            def qk(cc, base, dst):
                def emit():
                    ps = psB.tile([128, 512], F32, tag="b5", name="psQK")
                    for dk in range(NDK):
                        nc.tensor.matmul(
                            ps[:],
                            w_sb[:, dk * WCOL + base + cc * 128:
                                 dk * WCOL + base + cc * 128 + 128],
                            xc[:, dk, :],
                            start=(dk == 0), stop=(dk == NDK - 1))
                    nc.vector.tensor_copy(dst[cc][:, ts], ps[:])
                return emit

            def vv(tt):
                def emit():
                    ps = psB.tile([128, 512], F32, tag="b5", name="psV")
                    for dk in range(NDK):
                        nc.tensor.matmul(
                            ps[:, 0:256],
                            xc[:, dk, 128 * tt:128 * tt + 128],
                            w_sb[:, dk * WCOL + 512:dk * WCOL + 768],
                            start=(dk == 0), stop=(dk == NDK - 1))
                    ti = t5 * 4 + tt
                    nc.vector.tensor_copy(
                        v128s[b][:, ti * NH:(ti + 1) * NH, 64:128],
                        ps[:, 0:256].rearrange("p (n c) -> p n c", c=64))
                return emit

            return ([qk(cc, 0, qts[b]) for cc in range(2)]
                    + [qk(cc, 256, kts[b]) for cc in range(2)]
                    + [vv(tt) for tt in range(4)])

        def proj_groups(b, j, yts):
            """Projection fill groups for q-block (b, j): 8 psO tiles,
            each 2 matmuls + a direct PSUM->DRAM f32 DMA."""
            def po(tt, nn2):
                def emit():
                    ps = psB.tile([128, 512], F32, tag="b5", name="psO")
                    for ff in range(2):
                        nc.tensor.matmul(
                            ps[:],
                            yts[ff][:, 128 * tt:128 * tt + 128],
                            wp_sb[:, ff * D + 512 * nn2:ff * D + 512 * nn2 + 512],
                            start=(ff == 0), stop=(ff == 1))
                    ost = ostp.tile([128, 512], BF16, tag="o", name="ost")
                    nc.vector.tensor_copy(ost[:], ps[:])
                    nc.sync.dma_start(
                        out_d[b, 512 * j + 128 * tt:512 * j + 128 * tt + 128,
                              512 * nn2:512 * nn2 + 512],
                        ost[:])
                return emit
            return [po(tt, nn2) for tt in range(4) for nn2 in range(2)]

        prev = None   # (b, j, yts) of previous block
        for m in range(2 * NT5):
            b, j = divmod(m, NT5)
            nk = 4 * j + 4
            offs = [128 * (i - 4 * j) if i > 4 * j else 0 for i in range(nk)]

            if m + 2 < 2 * NT5:
                xc_dma(m + 2)
            # fill queue: phase A of block m+1 first (its deps are long
            # satisfied), then the projection of block m-1 (whose yt
            # divides drain on DVE during the first fills).
            fills = []
            if m + 1 < 2 * NT5:
                b2, j2 = divmod(m + 1, NT5)
                fills += phA_groups(b2, j2)
            if prev is not None:
                fills += proj_groups(*prev)
            if m == 0:
                # prologue: phase A chunk 0 emitted before anything else
                for g in phA_groups(0, 0):
                    g()

            # ---- S + exp for both head pairs, fills interleaved ----
            Ps = {}
            fi = 0
            for hp in range(2):
                qth, kth = qts[b][hp], kts[b][hp]
                for i in range(nk):
                    off = offs[i]
                    psS = psS_pool.tile([128, 1024], F32, tag="s", name="psS")
                    P = pP.tile([128, 1024], BF16, tag="p", name="P")
                    Ps[(hp, i)] = P
                    nc.tensor.matmul(
                        psS[:, off:512],
                        kth[0:64, 128 * i:128 * i + 128],
                        qth[0:64, 512 * j + off:512 * (j + 1)],
                        start=True, stop=True)
                    nc.tensor.matmul(
                        psS[:, 512:1024 - off],
                        kth[64:128, 128 * i:128 * i + 128],
                        qth[64:128, 512 * j + off:512 * (j + 1)],
                        start=True, stop=True)
                    nc.scalar.activation(
                        P[:, off:1024 - off], psS[:, off:1024 - off], EXP,
                        scale=0.125)
                    if i >= 4 * j:  # diagonal: causal triangle on both heads
                        nc.vector.tensor_mul(
                            P[:, off:off + 128], P[:, off:off + 128], tri[:])
                        nc.vector.tensor_mul(
                            P[:, 512:640], P[:, 512:640], tri[:])
                    # interleave fills in pairs every 2nd S tile: each
                    # row-tiled<->full-array transition exposes ~120ns of
                    # LDWEIGHTS, so group the S pairs to amortize it
                    if i % 2 == 1:
                        for g in fills[fi:fi + 2]:
                            g()
                        fi += 2
            while fi < len(fills):
                fills[fi]()
                fi += 1

            # ---- PV + normalize ----
            yt = [ytp.tile([128, 512], BF16, tag=f"yt{ff}", name=f"yt{ff}")
                  for ff in range(2)]
            for hp in range(2):
                for h01 in range(2):
                    h = 2 * hp + h01
                    psY = psB.tile([128, 512], F32, tag="b5", name="psY")
                    for i in range(nk):
                        off = offs[i]
                        mv = (Ps[(hp, i)][:, off:512] if h01 == 0
                              else Ps[(hp, i)][:, 512:1024 - off])
                        nc.tensor.matmul(
                            psY[:, off:512],
                            vs[b][:, 512 * i + 128 * h:512 * i + 128 * h + 128],
                            mv,
                            start=(i == 0), stop=(i == nk - 1))
                    rc = rcp.tile([1, 512], F32, tag="rc", name="rc")
                    nc.vector.reciprocal_approx_fast(rc[:], psY[0:1, :])
                    rb = rcp.tile([128, 512], F32, tag="rb", name="rb")
                    nc.gpsimd.partition_broadcast(rb[:], rc[:])
                    nc.vector.tensor_mul(
                        yt[hp][64 * h01:64 * h01 + 64, :],
                        psY[64:128, :], rb[64:128, :])
            prev = (b, j, yt)

        for g in proj_groups(*prev):   # epilogue: last block's projection
            g()

    nc.compile()
    return nc


def make_in_maps(x, W_qkv, W_proj):
    tri = np.triu(np.ones((128, 128), dtype=np.float32)).astype(ml_dtypes.bfloat16)
    xts = []
    for bg in range(2):
        xb = np.ascontiguousarray(x[2 * bg:2 * bg + 2]).astype(ml_dtypes.bfloat16)
        # [2, T, D] -> [2, NDK, 128, T]
        xts.append(np.ascontiguousarray(
            xb.reshape(NB, T, NDK, 128).transpose(0, 2, 3, 1)))
    in_maps = []
    for c in range(NC):
        bg, hg = c // 4, c % 4
        wq = np.concatenate(
            [W_qkv[:, 256 * hg:256 * hg + 256],
             W_qkv[:, 1024 + 256 * hg:1024 + 256 * hg + 256],
             W_qkv[:, 2048 + 256 * hg:2048 + 256 * hg + 256]], axis=1)
        in_maps.append({
            "xt": xts[bg],
            "wqkv": wq.astype(ml_dtypes.bfloat16),
            "wproj": W_proj[256 * hg:256 * hg + 256, :].astype(ml_dtypes.bfloat16),
            "tri": tri,
        })
    return in_maps


def kernel(x, W_qkv, W_proj):
    x = np.asarray(x, dtype=np.float32)
    W_qkv = np.asarray(W_qkv, dtype=np.float32)
    W_proj = np.asarray(W_proj, dtype=np.float32)
    nc = build()
    res = run_bass_kernel_spmd(nc, make_in_maps(x, W_qkv, W_proj), list(range(NC)))
    out = np.zeros((B, T, D), dtype=np.float64)
    for c in range(NC):
        bg = c // 4
        out[2 * bg:2 * bg + 2] += res.results[c]["out"].astype(np.float64)
    return out.astype(np.float32)
